# revision 20
# baseline (speedup 1.0000x reference)
"""CondConv3d kernel for 8 TRN2 NeuronCores (v2: unpadded 64-wide rows).

Math: the reference einsum 'bi,eocdwh->bocdwh' shares no index between
routing_weights and weight, so it factorizes:
    eff_kernel[b] = (sum_i routing[b,i]) * (sum_e weight[e])
    eff_bias[b]   = (sum_i routing[b,i]) * (sum_e bias[e])
=> out[b] = conv3d(x[b], s_b * W_sum, pad=1) + s_b * bias_sum

Sharding: data-parallel over batch B=8, one sample per core. The
per-sample scalar s_b is folded into that core's weights/bias on host.

Per-core kernel (bf16 in/out, fp32 PSUM accumulate):
  - x lives in SBUF UNPADDED: [96, 16*4096] bf16, partition groups
    [0,32)/[32,64)/[64,96) hold kd-shifted copies x(d-1)/x(d)/x(d+1),
    each loaded straight from HBM (a leading all-zero slice in the
    DRAM tensor supplies the depth pads). All DMAs are large contiguous
    copies (4 slices = 32KB per partition per transfer).
  - conv = 9 PSUM-accumulated matmuls per 512-output chunk; the kh/kw
    taps are free-dim address shifts of +-64/+-1. Boundary zero-padding
    is realized by RESTRICTING the matmul APs instead of padding data:
    kw=0 taps write out cols 1..63 only, kw=2 cols 0..62 (2-D APs with
    row stride 64); kh=0 skips out row 0 of each slice (chunk 0), kh=2
    skips row 63 (chunk 7). The first tap (kh=1,kw=1) is full-512 and
    carries the PSUM start flag.
  - 4 depth slices run concurrently via PE column tiling
    (tile_position=(0,32j)) so the PSUM/drain uses all 128 partitions.
  - drain: ScalarE/VectorE copy full contiguous [128,512] PSUM->SBUF
    bf16 with per-partition bias add; output streams out in two DMAs
    per 4-slice group so there is no large DMA tail.
"""

import sys

if "/opt/trn_rl_repo" not in sys.path:
    sys.path.insert(0, "/opt/trn_rl_repo")

import numpy as np
import ml_dtypes

import concourse.bass as bass
import concourse.tile as tile
from concourse import bacc, mybir
from concourse.bass_utils import run_bass_kernel_spmd

# problem shape (hardcoded per contest rules)
B, CI, CO, D, H, W = 8, 32, 32, 16, 64, 64
K = 3
NCORES = 8

SLOT = H * W            # 4096 elements per depth slice per partition
NSLICE = D + 1          # leading all-zero slice at index 0
VOL0 = 64               # front guard (never read; belt-and-suspenders)
NSTEP = 9
NCHUNK = 8              # 8 chunks of 512 = one 64x64 slice

# tap order: the full-rectangle (kh=1,kw=1) tap goes first so its
# start=True initializes every PSUM position of the chunk.
TAPS = [(1, 1), (1, 0), (1, 2), (0, 0), (0, 1), (0, 2), (2, 0), (2, 1), (2, 2)]

F32 = mybir.dt.float32
BF16 = mybir.dt.bfloat16

_CACHE = {}


def _build_nc():
    # Bacc (vs raw Bass) runs the wait-fixup passes: an ISA instruction can
    # carry only 1 semaphore wait; Bacc spills extras to ldweights/events.
    nc = bacc.Bacc(None)
    x_d = nc.declare_dram_parameter("x", [CI, NSLICE, SLOT], BF16, isOutput=False)
    w_d = nc.declare_dram_parameter("w", [96, NSTEP * CO], BF16, isOutput=False)
    b_d = nc.declare_dram_parameter("bias", [128, 1], F32, isOutput=False)
    o_d = nc.declare_dram_parameter("out", [CO, D, H * W], BF16, isOutput=True)

    with tile.TileContext(nc) as tc:
        with (
            tc.tile_pool(name="const", bufs=1) as const,
            tc.tile_pool(name="outs", bufs=3) as outp,
            tc.tile_pool(name="psum", bufs=8, space="PSUM") as psump,
        ):
            xp = const.tile([96, VOL0 + D * SLOT + 64], BF16)
            wsb = const.tile([96, NSTEP, CO], BF16)
            bsb = const.tile([128, 1], F32)

            nc.sync.dma_start(
                out=wsb[:, :, :],
                in_=w_d[:].rearrange("p (s o) -> p s o", s=NSTEP),
            )
            nc.sync.dma_start(out=bsb[:, :], in_=b_d[:])
            nc.vector.memset(xp[:, :VOL0], 0.0)
            nc.vector.memset(xp[:, VOL0 + D * SLOT :], 0.0)

            def fill(q):
                """Load slots 4q..4q+3 of all three kd-shifted groups.

                x_d slice k holds x[k-1] (k=0 is the zero slice), so group
                g's slot d wants x[d+g-1] = x_d slice d+g. Batching 4 slices
                per DMA keeps the sync engine's descriptor-dispatch cost off
                the critical path.
                """
                steps = []
                for h in (0, 1, 2, 3):
                    d = 4 * q + h
                    o0 = VOL0 + d * SLOT
                    dst = xp[:, o0 : o0 + SLOT]

                    def step(d=d, dst=dst):
                        nc.sync.dma_start(
                            out=dst[32:64], in_=x_d[:, d + 1, :]
                        )
                        if d == 0:
                            # slot 0 of g0 is x[-1] = zeros: memset beats
                            # an HBM read of the zero slice
                            nc.gpsimd.memset(dst[0:32], 0.0)
                        else:
                            nc.sync.dma_start(
                                out=dst[0:32], in_=x_d[:, d, :]
                            )
                        if d < 15:
                            nc.sync.dma_start(
                                out=dst[64:96], in_=x_d[:, d + 2, :]
                            )
                        else:
                            # d=15 wants x[16] = zeros (memset on the idle
                            # GpSimd engine, no HBM read)
                            nc.gpsimd.memset(dst[64:96], 0.0)

                    steps.append(step)
                return steps

            def compute_group0(prefetch=()):
                # Per-band PSUM for the first group: band j only needs the
                # three kd-copies of slice j (one fill step, ~0.5MB), so the
                # PE starts ~15us earlier than waiting for all four slices.
                ob = outp.tile([128, H * W], BF16)
                for j in range(4):
                    band = slice(32 * j, 32 * j + 32)
                    for c8 in range(NCHUNK):
                        if c8 == 4 and j < len(prefetch):
                            prefetch[j]()
                        ps = psump.tile([128, 512], F32)
                        ps3 = ps[:, :].rearrange("p (h w) -> p h w", h=8)
                        for t, (kh, kw) in enumerate(TAPS):
                            r0, r1 = 0, 8
                            if kh == 0 and c8 == 0:
                                r0 = 1
                            if kh == 2 and c8 == NCHUNK - 1:
                                r1 = 7
                            a = VOL0 + j * SLOT + c8 * 512 + (kh - 1) * 64
                            if kw == 1 and r0 == 0 and r1 == 8:
                                out_ap = ps[band, :]
                                rhs = xp[0:96, a : a + 512]
                            else:
                                v3 = xp[0:96, a : a + 512].rearrange(
                                    "p (h w) -> p h w", h=8
                                )
                                if kw == 1:
                                    out_ap = ps3[band, r0:r1, :]
                                    rhs = v3[:, r0:r1, :]
                                elif kw == 0:
                                    out_ap = ps3[band, r0:r1, 1:64]
                                    rhs = v3[:, r0:r1, 0:63]
                                else:
                                    out_ap = ps3[band, r0:r1, 0:63]
                                    rhs = v3[:, r0:r1, 1:64]
                            nc.tensor.matmul(
                                out=out_ap,
                                lhsT=wsb[0:96, 3 * kh + kw, :],
                                rhs=rhs,
                                start=(t == 0),
                                stop=(t == NSTEP - 1),
                                tile_position=(0, 32 * j),
                                skip_group_check=True,
                            )
                        dst3 = ob[band, c8 * 512 : (c8 + 1) * 512]
                        if c8 % 2 == 0:
                            nc.scalar.activation(
                                out=dst3,
                                in_=ps[band, :],
                                func=mybir.ActivationFunctionType.Identity,
                                bias=bsb[band, :],
                                scale=1.0,
                            )
                        else:
                            nc.vector.tensor_scalar_add(
                                dst3, ps[band, :], bsb[band, :]
                            )
                        # out columns are complete once the last band wrote
                        if j == 3 and c8 in (1, 3, 5, 7):
                            hi = (c8 + 1) * 512
                            lo = hi - 1024
                            dst = bass.AP(
                                tensor=o_d,
                                offset=lo,
                                ap=[
                                    [H * W, 4],
                                    [D * H * W, CO],
                                    [1, hi - lo],
                                ],
                            )
                            nc.sync.dma_start(out=dst, in_=ob[:, lo:hi])

            def compute_group(g, prefetch=()):
                ob = outp.tile([128, H * W], BF16)
                for c8 in range(NCHUNK):
                    if c8 < len(prefetch):
                        prefetch[c8]()
                    # one full PSUM bank = 8 output rows of 64
                    ps = psump.tile([128, 512], F32)
                    ps3 = ps[:, :].rearrange("p (h w) -> p h w", h=8)
                    for t, (kh, kw) in enumerate(TAPS):
                        # slice-edge rows whose kh tap would cross into the
                        # neighboring depth slice are simply not written
                        r0, r1 = 0, 8
                        if kh == 0 and c8 == 0:
                            r0 = 1
                        if kh == 2 and c8 == NCHUNK - 1:
                            r1 = 7
                        for j in range(4):
                            d = 4 * g + j
                            a = VOL0 + d * SLOT + c8 * 512 + (kh - 1) * 64
                            band = slice(32 * j, 32 * j + 32)
                            if kw == 1 and r0 == 0 and r1 == 8:
                                out_ap = ps[band, :]
                                rhs = xp[0:96, a : a + 512]
                            else:
                                v3 = xp[0:96, a : a + 512].rearrange(
                                    "p (h w) -> p h w", h=8
                                )
                                if kw == 1:
                                    out_ap = ps3[band, r0:r1, :]
                                    rhs = v3[:, r0:r1, :]
                                elif kw == 0:
                                    out_ap = ps3[band, r0:r1, 1:64]
                                    rhs = v3[:, r0:r1, 0:63]
                                else:  # kw == 2
                                    out_ap = ps3[band, r0:r1, 0:63]
                                    rhs = v3[:, r0:r1, 1:64]
                            nc.tensor.matmul(
                                out=out_ap,
                                lhsT=wsb[0:96, 3 * kh + kw, :],
                                rhs=rhs,
                                start=(t == 0),
                                stop=(t == NSTEP - 1),
                                tile_position=(0, 32 * j),
                                # sim's group tracker is bank-coarse; the
                                # 4 col-tiles run disjoint partition ranges
                                skip_group_check=True,
                            )
                    # drain PSUM -> SBUF bf16 with bias add (contiguous 512)
                    dst3 = ob[:, c8 * 512 : (c8 + 1) * 512]
                    if c8 % 2 == 0:
                        nc.scalar.activation(
                            out=dst3,
                            in_=ps[:, :],
                            func=mybir.ActivationFunctionType.Identity,
                            bias=bsb[:, :],
                            scale=1.0,
                        )
                    else:
                        nc.vector.tensor_scalar_add(dst3, ps[:, :], bsb[:, :])
                    # stream finished rows out so there is no big DMA tail;
                    # pieces get finer toward the end so the final transfer
                    # (which serializes after the last drain) is small
                    if g == 3:
                        splits = {1: 1024, 3: 1024, 5: 1024, 6: 512, 7: 512}
                    else:
                        splits = {1: 1024, 3: 1024, 5: 1024, 7: 1024}
                    if c8 in splits:
                        step = splits[c8]
                        hi = (c8 + 1) * 512
                        lo = hi - step
                        dst = bass.AP(
                            tensor=o_d,
                            offset=4 * g * (H * W) + lo,
                            ap=[
                                [H * W, 4],       # j (slice within group)
                                [D * H * W, CO],  # o (channel)
                                [1, hi - lo],
                            ],
                        )
                        nc.sync.dma_start(out=dst, in_=ob[:, lo:hi])

            for step in fill(0):
                step()
            compute_group0(prefetch=fill(1))
            for g in range(1, 4):
                nxt = fill(g + 1) if g + 1 < 4 else ()
                compute_group(g, prefetch=nxt)

    nc.finalize()  # Bacc: runs wait-spill + register allocation passes
    return nc


def _get_nc():
    if "nc" not in _CACHE:
        _CACHE["nc"] = _build_nc()
    return _CACHE["nc"]


def _host_prep(x, routing_weights, weight, bias):
    """Build the per-core input maps (one batch sample per core)."""
    x = np.asarray(x, dtype=np.float32)
    routing_weights = np.asarray(routing_weights, dtype=np.float32)
    weight = np.asarray(weight, dtype=np.float32)
    bias = np.asarray(bias, dtype=np.float32)

    s = routing_weights.sum(axis=1)          # [B]
    w_sum = weight.sum(axis=0)               # [CO, CI, K, K, K]
    b_sum = bias.sum(axis=0)                 # [CO]

    # lhsT layout: [p=(kd,ci), (kh,kw), o]
    wt = np.transpose(w_sum, (2, 1, 3, 4, 0)).reshape(96, NSTEP * CO)

    # slice 0 = zeros (depth pad); slice k = x[k-1]; no spatial padding
    xz = np.zeros((B, CI, NSLICE, SLOT), dtype=np.float32)
    xz[:, :, 1:, :] = x.reshape(B, CI, D, SLOT)

    in_maps = []
    for b in range(B):
        wb = (s[b] * wt).astype(ml_dtypes.bfloat16)
        bb = np.tile(s[b] * b_sum, 4).reshape(128, 1).astype(np.float32)
        in_maps.append(
            {
                "x": np.ascontiguousarray(xz[b].astype(ml_dtypes.bfloat16)),
                "w": np.ascontiguousarray(wb),
                "bias": bb,
            }
        )
    return in_maps


def kernel(x, routing_weights, weight, bias):
    in_maps = _host_prep(x, routing_weights, weight, bias)
    nc = _get_nc()
    _CACHE["last_in_maps"] = in_maps
    res = run_bass_kernel_spmd(nc, in_maps, list(range(NCORES)))
    _CACHE["last_result"] = res
    out = np.stack(
        [
            np.asarray(res.results[b]["out"]).astype(np.float32).reshape(
                CO, D, H, W
            )
            for b in range(B)
        ]
    )
    return out
